# revision 26
# baseline (speedup 1.0000x reference)
"""Trainium2 Bass kernel for nn_CayleyFilter (gnn_message_passing).

Math: the reference's Jacobi step degenerates — its SpMM terms cancel
algebraically:
    tr = (offr + diag*zr) + zi - offr == diag*zr + zi   (+- fp rounding noise)
    ti = (offi + diag*zi) - zr - offi == diag*zi - zr
so each Cayley iteration is an elementwise multiply by the per-node
unit-modulus complex scalar s_p = (d_p - i)^2 / (d_p^2 + 1).  Hence
    z_k = s^k (x)   and the whole module collapses to one GEMM:
    out[(n,p), o] = sum_{g,c} coef_g[p] * x[n,c,p] * W2[(g,c), o]
with coef_g in {Re(s^k), Im(s^k)} (18 groups) and W2 = 2*[Wr; Wi].

FAST PATH (detected at runtime): diag(L) of a normalized Laplacian is
exactly all-ones, so s_p = -i for every node and the coef vectors are
CONSTANT over p (cycling 1, 0, -1, 0).  Constant coefs fold into the
weights on the host:  out[n,p,:] = x[n,:,p]^T @ Weff,  Weff[32, 64].
Device kernel: pack 2 batches per matmul (contraction 64 = 2x32 with
block-diagonal Weff, output 128 = 2x64 partitions), fp16 single-pass
matmuls into fp16 PSUM banks, DVE/ACT copies to SBUF, fp16 DMA out.
This is at the PE output-bandwidth floor (128 rows per column-cycle).

GENERAL PATH (any diag_L): the original coefficient-weighted GEMM with
on-device feature build (kept as a correctness fallback; see git/session
history for its design notes).
"""

import numpy as np

N, C, M, MSIDE, COUT, ORDER = 32, 32, 4096, 64, 64, 8
NCORES = 8
NLOC = N // NCORES            # 4 batches per core
KTOT = ORDER + 1              # 9
NGRP = 2 * KTOT               # 18 coefficient groups (real k, imag k)
NCHUNK = 5                    # contraction chunks of 128 rows (640 padded)
PT = 512                      # p tile (matmul moving free dim, fp32 max)
NPT = M // PT                 # 8

# fast path tiling
NPAIR = NLOC // 2             # 2 batch-pairs per core
FPT = 512                     # moving free dim (one fp32 PSUM bank)
FNPT = M // FPT               # 8

_STATE = {}
LAST_RESULTS = None


# --------------------------------------------------------------------------
# fast path: constant coefs folded into Weff on host; tiny GEMM on device
# --------------------------------------------------------------------------

def _make_nc_fast(loop_reps=0, ogrp=4, split_x0=1024, ps_bufs=6, o_bufs=4,
                  order=None, pool_dma=0, cgrp=1, last_split=0, groups=None,
                  copy_pat=(1, 0, 0)):
    # copy_pat=(1,0,0): 1/3 of PSUM->SBUF copies on DVE, 2/3 on ACT.
    # ScalarE is measurably cheaper per PSUM copy on real silicon (it sits
    # closer to PSUM); pure-ACT and DVE-heavy splits both bench worse.
    """out[(pair),(b2,o),p] = blockdiag(Weff,Weff).T @ xpair[(b2,c),p].

    ogrp: matmul/copy tiles (FPT cols each) gathered into one out-DMA.
    cgrp: matmul tiles per PSUM->SBUF copy (PSUM tile spans cgrp banks).
    Copies alternate DVE / ACT.  pool_dma: every pool_dma-th out-DMA issues
    via GpSimd/SWDGE instead of SP/HWDGE (0 = all on SP; crashes current
    walrus — keep 0).  loop_reps>0 wraps everything (including input DMA)
    in a hardware For_i loop for steady-state benching.
    """
    import contextlib

    import concourse.bass as bass
    import concourse.mybir as mybir
    from concourse.tile import TileContext

    f16 = mybir.dt.float16
    f32 = mybir.dt.float32

    nc = bass.Bass()
    xp_d = nc.dram_tensor("xp", [NPAIR, 64, M], f16, kind="ExternalInput")
    w_d = nc.dram_tensor("w", [64, 2 * COUT], f16, kind="ExternalInput")
    out_d = nc.dram_tensor("out", [NPAIR, 2 * COUT, M], f16, kind="ExternalOutput")

    with TileContext(nc) as tc:
        with (
            tc.tile_pool(name="const", bufs=1) as cpool,
            tc.tile_pool(name="osb", bufs=o_bufs) as opool,
            tc.tile_pool(name="ps", bufs=ps_bufs, space="PSUM") as pspool,
        ):
            loop_cm = tc.For_i(0, loop_reps, 1) if loop_reps else contextlib.nullcontext()
            with loop_cm:
                w_sb = cpool.tile([64, 2 * COUT], f16, tag="w")
                xs = [cpool.tile([64, M], f16, tag=f"x{j}", name=f"x{j}")
                      for j in range(NPAIR)]
                # named load units; `order` picks emission sequence
                loads = {
                    "w": lambda: nc.sync.dma_start(w_sb[:, :], w_d[:]),
                    "x0": lambda: nc.sync.dma_start(xs[0][:, :], xp_d[0]),
                    "x0a": lambda: nc.sync.dma_start(
                        xs[0][:, :split_x0], xp_d[0, :, :split_x0]),
                    "x0b": lambda: nc.sync.dma_start(
                        xs[0][:, split_x0:], xp_d[0, :, split_x0:]),
                    "x1": lambda: nc.sync.dma_start(xs[1][:, :], xp_d[1]),
                    "x1a": lambda: nc.sync.dma_start(
                        xs[1][:, :M // 2], xp_d[1, :, :M // 2]),
                    "x1b": lambda: nc.sync.dma_start(
                        xs[1][:, M // 2:], xp_d[1, :, M // 2:]),
                }
                for name in (order or ("w", "x0a", "x0b", "x1")):
                    loads[name]()

                ti = 0
                oi = 0
                if groups is None:
                    gpat = [ogrp] * (FNPT // ogrp)
                else:
                    gpat = list(groups)
                    assert sum(gpat) == FNPT
                ngroups = len(gpat) * NPAIR
                for j in range(NPAIR):
                    pt0 = 0
                    for gw in gpat:
                        OW = gw * FPT
                        osb = opool.tile([2 * COUT, OW], f16)
                        for cg in range(gw // cgrp):
                            ps = pspool.tile([2 * COUT, cgrp * FPT], f32)
                            for h in range(cgrp):
                                pt = pt0 + cg * cgrp + h
                                psl = slice(pt * FPT, (pt + 1) * FPT)
                                nc.tensor.matmul(
                                    ps[:, h * FPT:(h + 1) * FPT],
                                    w_sb[:, :], xs[j][:, psl],
                                    start=True, stop=True,
                                )
                            csl = slice(cg * cgrp * FPT, (cg + 1) * cgrp * FPT)
                            if copy_pat[ti % len(copy_pat)]:
                                nc.vector.tensor_copy(osb[:, csl], ps[:, :])
                            else:
                                nc.scalar.copy(osb[:, csl], ps[:, :])
                            ti += 1
                        oi += 1
                        eng = (nc.gpsimd if (pool_dma and oi % pool_dma == 0)
                               else nc.sync)
                        dst = out_d[j, :, pt0 * FPT: pt0 * FPT + OW]
                        if last_split and oi == ngroups and OW > last_split:
                            h = OW - last_split
                            eng.dma_start(dst[:, :h], osb[:, :h])
                            eng.dma_start(dst[:, h:], osb[:, h:])
                        else:
                            eng.dma_start(dst, osb[:, :])
                        pt0 += gw

    import bass_rust
    bass_rust.generate_event_semaphores(nc)
    return nc


def _fold_weights(coefs, real_weights, imag_weights):
    """coefs: (NGRP, M) float64; returns Weff (C, COUT) float64 if the coef
    vectors are constant across nodes, else None."""
    span = coefs.max(axis=1) - coefs.min(axis=1)
    if span.max() > 1e-9:
        return None
    cvec = coefs[:, 0]                       # (18,)
    wr = np.asarray(real_weights, dtype=np.float64).reshape(KTOT, C, COUT)
    wi = np.asarray(imag_weights, dtype=np.float64).reshape(KTOT, C, COUT)
    weff = np.zeros((C, COUT), dtype=np.float64)
    for k in range(KTOT):
        weff += cvec[k] * wr[k] + cvec[KTOT + k] * wi[k]
    return 2.0 * weff


def _run_fast(x3, weff):
    global LAST_RESULTS
    from concourse.bass_utils import run_bass_kernel_spmd

    if "nc_fast" not in _STATE:
        _STATE["nc_fast"] = _make_nc_fast()
    nc = _STATE["nc_fast"]

    # blockdiag(Weff, Weff): rows 0-31 -> outs 0-63, rows 32-63 -> 64-127
    wblk = np.zeros((64, 2 * COUT), dtype=np.float16)
    wblk[:C, :COUT] = weff
    wblk[C:, COUT:] = weff
    x16 = x3.astype(np.float16).reshape(NCORES, NPAIR, 2 * C, M)

    in_maps = [{"xp": x16[i], "w": wblk} for i in range(NCORES)]
    res = run_bass_kernel_spmd(nc, in_maps, list(range(NCORES)))
    LAST_RESULTS = res

    out = np.empty((N, M, COUT), dtype=np.float32)
    for i in range(NCORES):
        o = res.results[i]["out"]            # (NPAIR, 128, M) f16
        for j in range(NPAIR):
            for h in range(2):
                out[NLOC * i + 2 * j + h] = o[j, COUT * h: COUT * (h + 1)].T
    return out.reshape(N, MSIDE, MSIDE, COUT)


# --------------------------------------------------------------------------
# general path (arbitrary diag_L): on-device feature build + 640-row GEMM
# --------------------------------------------------------------------------

def _make_nc(loop_reps=0, dve_chunks=(3, 4, 3), ft_bufs=4, psx_bufs=3):
    """Build the general SPMD program.

    loop_reps>0 wraps the compute in a hardware For_i loop (benchmark-only).
    dve_chunks: chunks built on VectorE per (n,pt), alternating by pt parity;
    the rest go to GpSimdE.
    """
    import contextlib

    import concourse.bass as bass
    import concourse.mybir as mybir
    from concourse.tile import TileContext

    f32 = mybir.dt.float32
    f32r = mybir.dt.float32r

    nc = bass.Bass()
    x_d = nc.dram_tensor("x", [NLOC, C, M], f32, kind="ExternalInput")
    coef_d = nc.dram_tensor("coef", [NCHUNK, 128, M], f32, kind="ExternalInput")
    w_d = nc.dram_tensor("w", [NCHUNK, 128, COUT], f32, kind="ExternalInput")
    out_d = nc.dram_tensor("out", [NLOC, COUT, M], f32, kind="ExternalOutput")

    with TileContext(nc) as tc:
        with (
            tc.tile_pool(name="const", bufs=1) as cpool,
            tc.tile_pool(name="wstage", bufs=1) as wpool,
            tc.tile_pool(name="ft", bufs=ft_bufs) as ftpool,
            tc.tile_pool(name="osb", bufs=4) as opool,
            tc.tile_pool(name="ps", bufs=3, space="PSUM") as pspool,
            tc.tile_pool(name="psx", bufs=psx_bufs, space="PSUM") as psxpool,
        ):
            xrep = [cpool.tile([128, M], f32, tag=f"xrep{n}", name=f"xr{n}")
                    for n in range(NLOC)]
            coef_sb = cpool.tile([128, NCHUNK * M], f32, tag="coef")

            w_f32 = wpool.tile([128, NCHUNK * COUT], f32, tag="wf32")
            w_sb = cpool.tile([128, NCHUNK * COUT], f32r, tag="w")
            nc.sync.dma_start(
                w_f32[:, :].rearrange("p (q o) -> p q o", q=NCHUNK),
                w_d[:].rearrange("q p o -> p q o"),
            )
            nc.vector.tensor_copy(w_sb[:, :], w_f32[:, :])

            CBW = M // NLOC
            for j in range(4):
                nc.sync.dma_start(xrep[0][32 * j:32 * (j + 1), :], x_d[0])
            for b in range(NLOC):
                for q in range(NCHUNK):
                    nc.sync.dma_start(
                        coef_sb[:, q * M + b * CBW: q * M + (b + 1) * CBW],
                        coef_d[q, :, b * CBW:(b + 1) * CBW],
                    )
                if b + 1 < NLOC:
                    n = b + 1
                    for j in range(4):
                        nc.sync.dma_start(xrep[n][32 * j:32 * (j + 1), :], x_d[n])

            loop_cm = tc.For_i(0, loop_reps, 1) if loop_reps else contextlib.nullcontext()
            with loop_cm:
                for n in range(NLOC):
                    for pt in range(NPT):
                        ndve = dve_chunks[pt % len(dve_chunks)]
                        ngp = NCHUNK - ndve
                        psl = slice(pt * PT, (pt + 1) * PT)

                        psx = psxpool.tile([128, PT], f32)
                        nc.scalar.copy(psx[:, :], xrep[n][:, psl])

                        ft = ftpool.tile([128, NCHUNK * PT], f32r)
                        coef3d = coef_sb[:, :].rearrange("r (q m) -> r q m", q=NCHUNK)
                        nc.vector.tensor_mul(
                            ft[:, : ndve * PT].rearrange(
                                "r (q p) -> r q p", q=ndve
                            ),
                            psx[:, :].unsqueeze(1).broadcast_to(
                                [128, ndve, PT]
                            ),
                            coef3d[:, :ndve, psl],
                        )
                        nc.gpsimd.tensor_mul(
                            ft[:, ndve * PT:].rearrange(
                                "r (q p) -> r q p", q=ngp
                            ),
                            xrep[n][:, psl].unsqueeze(1).broadcast_to(
                                [128, ngp, PT]
                            ),
                            coef3d[:, ndve:, psl],
                        )

                        ps = pspool.tile([COUT, PT], f32)
                        for q in range(NCHUNK):
                            nc.tensor.matmul(
                                ps[:, :],
                                w_sb[:, q * COUT:(q + 1) * COUT],
                                ft[:, q * PT:(q + 1) * PT],
                                start=(q == 0),
                                stop=(q == NCHUNK - 1),
                            )
                        osb = opool.tile([COUT, PT], f32)
                        nc.scalar.copy(osb[:, :], ps[:, :])
                        nc.sync.dma_start(out_d[n, :, psl], osb[:, :])

    import bass_rust
    bass_rust.generate_event_semaphores(nc)
    return nc


def _coefs_from_diag(diag_L):
    d = np.asarray(diag_L, dtype=np.float64)
    s = (d - 1j) ** 2 / (d * d + 1.0)
    coefs = np.empty((NGRP, M), dtype=np.float64)
    ck = np.ones(M, dtype=np.complex128)
    for k in range(KTOT):
        coefs[k] = ck.real
        coefs[KTOT + k] = ck.imag
        ck = ck * s
    return coefs


def _prep_host(x, real_weights, imag_weights, coefs):
    x = np.ascontiguousarray(np.asarray(x, dtype=np.float32).reshape(N, C, M))
    wr = np.asarray(real_weights, dtype=np.float32)
    wi = np.asarray(imag_weights, dtype=np.float32)

    coef_pad = np.zeros((NCHUNK * 128, M), dtype=np.float32)
    coef_pad[:NGRP * C] = np.repeat(coefs.astype(np.float32), C, axis=0)
    w_pad = np.zeros((NCHUNK * 128, COUT), dtype=np.float32)
    w_pad[: KTOT * C] = 2.0 * wr
    w_pad[KTOT * C: NGRP * C] = 2.0 * wi
    return (
        x,
        np.ascontiguousarray(coef_pad.reshape(NCHUNK, 128, M)),
        np.ascontiguousarray(w_pad.reshape(NCHUNK, 128, COUT)),
    )


def kernel(x, real_weights, imag_weights, diag_L, vals, rows, cols):
    global LAST_RESULTS
    import os

    from concourse.bass_utils import run_bass_kernel_spmd

    coefs = _coefs_from_diag(diag_L)
    x3 = np.ascontiguousarray(np.asarray(x, dtype=np.float32).reshape(N, C, M))

    if not os.environ.get("FORCE_GENERAL_PATH"):
        weff = _fold_weights(coefs, real_weights, imag_weights)
        if weff is not None:
            return _run_fast(x3, weff)

    x3, coef, w = _prep_host(x, real_weights, imag_weights, coefs)

    if "nc" not in _STATE:
        _STATE["nc"] = _make_nc()
    nc = _STATE["nc"]

    in_maps = [
        {"x": x3[NLOC * i: NLOC * (i + 1)], "coef": coef, "w": w}
        for i in range(NCORES)
    ]
    res = run_bass_kernel_spmd(nc, in_maps, list(range(NCORES)))
    LAST_RESULTS = res

    out = np.empty((N, M, COUT), dtype=np.float32)
    for i in range(NCORES):
        o = res.results[i]["out"]            # (NLOC, COUT, M)
        for j in range(NLOC):
            out[NLOC * i + j] = o[j].T
    return out.reshape(N, MSIDE, MSIDE, COUT)
